# revision 1
# baseline (speedup 1.0000x reference)
"""GQA attention block on 8 Trainium2 cores.

Sharding: data-parallel over batch B=2 x tensor-parallel over the 4 KV groups
(cores 0-3 -> batch 0 groups 0-3, cores 4-7 -> batch 1 groups 0-3).
Each core computes Q/K/V projections for its group, attention for its 4 query
heads, and a row-sharded partial of the output projection.  The host sums the
4 partials per batch and adds the output bias.

All matmuls run in float32r (full-rate fp32 mode on the PE array);
accumulation is fp32 in PSUM.

On-device layout trick: the host feeds x pre-transposed (xT = x[b].T), so
every projection matmul can consume it directly as the moving operand with
the contraction dim (d_model) on partitions -- no on-device transposes except
16 small 128x128 PE transposes to turn V^T into V.
"""
import sys

sys.path.insert(0, "/opt/trn_rl_repo")

import math
from contextlib import ExitStack

import numpy as np

import concourse.bacc as bacc
import concourse.tile as tile
import concourse.mybir as mybir
from concourse.bass_utils import run_bass_kernel_spmd
from concourse.masks import make_identity

F32 = mybir.dt.float32
F32R = mybir.dt.float32r
AF = mybir.ActivationFunctionType

D = 2048          # d_model
S = 2048          # sequence length
HD = 128          # head dim
R = 4             # q heads per kv group (on one core)
GD = R * HD       # 512: q-projection width per core
KT_TILES = S // 128   # 16 key-time tiles
KD_TILES = D // 128   # 16 contraction tiles for projections
N_SC = 4          # s-chunks of 512
SC = S // N_SC    # 512
SCALE = 1.0 / math.sqrt(HD)

_CACHED = {}


def _build():
    nc = bacc.Bacc("TRN2", target_bir_lowering=False, debug=False, num_devices=8)

    XT = nc.dram_tensor("xt", [D, S], F32R, kind="ExternalInput")
    WQ = nc.dram_tensor("wq", [D, GD], F32R, kind="ExternalInput")
    WK = nc.dram_tensor("wk", [D, HD], F32R, kind="ExternalInput")
    WV = nc.dram_tensor("wv", [D, HD], F32R, kind="ExternalInput")
    WO = nc.dram_tensor("wo", [GD, D], F32R, kind="ExternalInput")
    BQ = nc.dram_tensor("bq", [128, R], F32, kind="ExternalInput")
    BK = nc.dram_tensor("bk", [128, 1], F32, kind="ExternalInput")
    BV = nc.dram_tensor("bv", [128, 1], F32, kind="ExternalInput")
    OUT = nc.dram_tensor("out", [S, D], F32, kind="ExternalOutput")

    with tile.TileContext(nc) as tc, ExitStack() as ctx:
        # ---- long-lived tiles ----
        lp = ctx.enter_context(tc.tile_pool(name="long", bufs=1))
        qt_sb = lp.tile([128, R, S], F32R)        # Q^T per head: [dq, h, s]
        kt_sb = lp.tile([128, S], F32R)           # K^T: [dk, t]
        vt_sb = lp.tile([128, S], F32R)           # V^T: [dv, t]
        v_sb = lp.tile([128, KT_TILES, HD], F32R) # V natural: [t_sub, t_tile, dv]
        bq_sb = lp.tile([128, R], F32)
        bk_sb = lp.tile([128, 1], F32)
        bv_sb = lp.tile([128, 1], F32)
        ones_col = lp.tile([128, 1], F32R)
        ones_row = lp.tile([1, 128], F32R)
        ident = lp.tile([128, 128], F32R)

        nc.sync.dma_start(bq_sb[:], BQ.ap())
        nc.sync.dma_start(bk_sb[:], BK.ap())
        nc.sync.dma_start(bv_sb[:], BV.ap())

        tmp_f = lp.tile([128, 128], F32)
        nc.gpsimd.memset(tmp_f[:], 1.0)
        nc.vector.tensor_copy(ones_col[:], tmp_f[:, 0:1])
        nc.vector.tensor_copy(ones_row[:], tmp_f[0:1, 0:128])
        make_identity(nc, tmp_f[:])
        nc.vector.tensor_copy(ident[:], tmp_f[:])

        # ---- phase A: projections ----
        with ExitStack() as actx:
            wp = actx.enter_context(tc.tile_pool(name="wqkv", bufs=1))
            xp = actx.enter_context(tc.tile_pool(name="xt", bufs=2))
            psa = actx.enter_context(tc.tile_pool(name="psa", bufs=4, space="PSUM"))

            wq_sb = wp.tile([128, KD_TILES, GD], F32R)
            wk_sb = wp.tile([128, KD_TILES, HD], F32R)
            wv_sb = wp.tile([128, KD_TILES, HD], F32R)
            wq_r = WQ.ap().rearrange("(ko p) n -> p ko n", p=128)
            wk_r = WK.ap().rearrange("(ko p) n -> p ko n", p=128)
            wv_r = WV.ap().rearrange("(ko p) n -> p ko n", p=128)

            for sc in range(N_SC):
                xt = xp.tile([128, KD_TILES, SC], F32R, tag="xt")
                xt_r = XT.ap()[:, sc * SC:(sc + 1) * SC].rearrange(
                    "(ko p) s -> p ko s", p=128
                )
                # interleave per-k so the k=0 matmuls can start almost
                # immediately (weights ride along with the first chunk)
                for k in range(KD_TILES):
                    nc.sync.dma_start(xt[:, k, :], xt_r[:, k, :])
                    if sc == 0:
                        nc.sync.dma_start(wq_sb[:, k, :], wq_r[:, k, :])
                        nc.sync.dma_start(wk_sb[:, k, :], wk_r[:, k, :])
                        nc.sync.dma_start(wv_sb[:, k, :], wv_r[:, k, :])
                # Q^T for the 4 heads
                for dq in range(R):
                    ps = psa.tile([128, SC], F32, tag="psa")
                    for k in range(KD_TILES):
                        nc.tensor.matmul(
                            ps[:],
                            lhsT=wq_sb[:, k, dq * 128:(dq + 1) * 128],
                            rhs=xt[:, k, :],
                            start=(k == 0),
                            stop=(k == KD_TILES - 1),
                        )
                    nc.scalar.activation(
                        qt_sb[:, dq, sc * SC:(sc + 1) * SC], ps[:],
                        AF.Identity, bias=bq_sb[:, dq:dq + 1],
                    )
                # K^T
                ps = psa.tile([128, SC], F32, tag="psa")
                for k in range(KD_TILES):
                    nc.tensor.matmul(
                        ps[:], lhsT=wk_sb[:, k, :], rhs=xt[:, k, :],
                        start=(k == 0), stop=(k == KD_TILES - 1),
                    )
                nc.scalar.activation(
                    kt_sb[:, sc * SC:(sc + 1) * SC], ps[:],
                    AF.Identity, bias=bk_sb[:],
                )
                # V^T
                ps = psa.tile([128, SC], F32, tag="psa")
                for k in range(KD_TILES):
                    nc.tensor.matmul(
                        ps[:], lhsT=wv_sb[:, k, :], rhs=xt[:, k, :],
                        start=(k == 0), stop=(k == KD_TILES - 1),
                    )
                nc.scalar.activation(
                    vt_sb[:, sc * SC:(sc + 1) * SC], ps[:],
                    AF.Identity, bias=bv_sb[:],
                )

            # V^T -> V natural (16 PE transposes)
            pst = actx.enter_context(tc.tile_pool(name="pst", bufs=2, space="PSUM"))
            for t in range(KT_TILES):
                pt_ps = pst.tile([128, 128], F32R, tag="pst")
                nc.tensor.transpose(
                    pt_ps[:], vt_sb[:, t * 128:(t + 1) * 128], ident[:]
                )
                nc.vector.tensor_copy(v_sb[:, t, :], pt_ps[:])

        # ---- phase B: attention + out-proj ----
        with ExitStack() as bctx:
            wop = bctx.enter_context(tc.tile_pool(name="wo", bufs=1))
            wo_sb = wop.tile([128, R, D], F32R)
            nc.sync.dma_start(wo_sb[:], WO.ap().rearrange("(h p) n -> p h n", p=128))

            pss = bctx.enter_context(tc.tile_pool(name="pss", bufs=2, space="PSUM"))
            pso = bctx.enter_context(tc.tile_pool(name="pso", bufs=2, space="PSUM"))
            psm = bctx.enter_context(tc.tile_pool(name="psm", bufs=2, space="PSUM"))
            ptp = bctx.enter_context(tc.tile_pool(name="ptp", bufs=4))
            accp = bctx.enter_context(tc.tile_pool(name="accp", bufs=3))
            otp = bctx.enter_context(tc.tile_pool(name="otp", bufs=3))
            outp = bctx.enter_context(tc.tile_pool(name="outp", bufs=4))

            def t_loop(sc, h, ot_sb, mid_cb=None):
                """scores -> exp -> attn accumulation + partial denom sums.
                Returns state consumed later by tail().  mid_cb is emitted
                after group 2 (pipelines the previous head's tail here)."""
                ps_o = pso.tile([128, SC], F32, tag="pso", name="ps_o")
                parts = [accp.tile([128, SC], F32, tag=f"acc{j}", name=f"acc{j}")
                         for j in range(4)]
                for tg in range(KT_TILES // 2):
                    if tg == 2 and mid_cb is not None:
                        mid_cb()
                    ps_s = pss.tile([128, 2, SC], F32, tag="pss", name="ps_s")
                    for i in range(2):
                        t = tg * 2 + i
                        nc.tensor.matmul(
                            ps_s[:, i, :],
                            lhsT=kt_sb[:, t * 128:(t + 1) * 128],
                            rhs=qt_sb[:, h, sc * SC:(sc + 1) * SC],
                            start=True, stop=True,
                        )
                    pt = ptp.tile([128, 2, SC], F32R, tag="pt", name="pt")
                    nc.scalar.activation(pt[:], ps_s[:], AF.Exp, scale=SCALE)
                    for i in range(2):
                        t = tg * 2 + i
                        nc.tensor.matmul(
                            ps_o[:],
                            lhsT=v_sb[:, t, :],
                            rhs=pt[:, i, :],
                            start=(t == 0),
                            stop=(t == KT_TILES - 1),
                            skip_group_check=True,
                        )
                    # partial denominator sums: parts[i + 2*(tg>=4)]
                    for i in range(2):
                        j = i + 2 * (tg >= 4)
                        src = pt[:, i, :].bitcast(F32)
                        if tg in (0, 4):
                            nc.vector.tensor_copy(parts[j][:], src)
                        else:
                            nc.vector.tensor_add(parts[j][:], parts[j][:], src)
                    if tg == 3:  # early merge of first half (off critical path)
                        nc.vector.tensor_add(parts[0][:], parts[0][:], parts[1][:])
                return ps_o, parts

            def tail(sc, h, ot_sb, ps_o, parts):
                """denominator -> reciprocal -> broadcast -> normalize.
                ps_d/ps_b live inside one pss-tagged slot (bank sharing)."""
                acc_r = accp.tile([128, SC], F32R, tag="acc_r", name="acc_r")
                nc.vector.tensor_add(parts[2][:], parts[2][:], parts[3][:])
                nc.vector.tensor_add(acc_r[:], parts[0][:], parts[2][:])
                ps_d = psm.tile([1, SC], F32, tag="psm", name="ps_d")
                nc.tensor.matmul(
                    ps_d[:], lhsT=ones_col[:], rhs=acc_r[:], start=True, stop=True
                )
                recip = accp.tile([1, SC], F32, tag="recip", name="recip")
                nc.vector.reciprocal_approx_fast(recip[:], ps_d[:])
                recip_r = accp.tile([1, SC], F32R, tag="recip_r", name="recip_r")
                nc.vector.tensor_copy(recip_r[:], recip[:])
                ps_b = psm.tile([128, SC], F32, tag="psm", name="ps_b")
                nc.tensor.matmul(
                    ps_b[:], lhsT=ones_row[:], rhs=recip_r[:],
                    start=True, stop=True,
                )
                bc = accp.tile([128, SC], F32, tag="bc", name="bc")
                nc.scalar.copy(bc[:], ps_b[:])
                nc.vector.tensor_mul(ot_sb[:, h, :], ps_o[:], bc[:])

            def out_proj(sc, ot_sb):
                for st in range(SC // 128):
                    for oc in range(D // 512):
                        ps_f = psm.tile([128, 512], F32, tag="psm", name="ps_f")
                        for dv in range(R):
                            nc.tensor.matmul(
                                ps_f[:],
                                lhsT=ot_sb[:, dv, st * 128:(st + 1) * 128],
                                rhs=wo_sb[:, dv, oc * 512:(oc + 1) * 512],
                                start=(dv == 0),
                                stop=(dv == R - 1),
                            )
                        o_t = outp.tile([128, 512], F32, tag="out", name="o_t")
                        if (st + oc) % 2 == 0:
                            nc.scalar.copy(o_t[:], ps_f[:])
                        else:
                            nc.vector.tensor_copy(o_t[:], ps_f[:])
                        nc.sync.dma_start(
                            OUT.ap()[
                                sc * SC + st * 128: sc * SC + (st + 1) * 128,
                                oc * 512:(oc + 1) * 512,
                            ],
                            o_t[:],
                        )

            # software pipeline: tail(i-1) is emitted after t_loop(i) so the
            # PE never sits in-order behind the DVE denominator chain; the
            # out-proj of chunk sc is emitted after t_loop(sc+1, h=0).
            for sc in range(N_SC):
                ot_sb = otp.tile([128, R, SC], F32R, tag="ot", name="ot_sb")
                for h in range(R):
                    ps_o, parts = t_loop(sc, h, ot_sb)
                    tail(sc, h, ot_sb, ps_o, parts)
                out_proj(sc, ot_sb)

    nc.compile()
    return nc


def _get_nc():
    if "nc" not in _CACHED:
        _CACHED["nc"] = _build()
    return _CACHED["nc"]


def _make_in_maps(x, Wq, bq, Wk, bk, Wv, bv, Wo):
    in_maps = []
    xts = [np.ascontiguousarray(x[b].T) for b in range(2)]
    for core in range(8):
        b, g = divmod(core, 4)
        in_maps.append({
            "xt": xts[b],
            "wq": np.ascontiguousarray(Wq[:, g * GD:(g + 1) * GD]),
            "wk": np.ascontiguousarray(Wk[:, g * HD:(g + 1) * HD]),
            "wv": np.ascontiguousarray(Wv[:, g * HD:(g + 1) * HD]),
            "wo": np.ascontiguousarray(Wo[g * GD:(g + 1) * GD, :]),
            "bq": np.ascontiguousarray(
                bq[g * GD:(g + 1) * GD].reshape(R, 128).T
            ),
            "bk": bk[g * HD:(g + 1) * HD].reshape(HD, 1).copy(),
            "bv": bv[g * HD:(g + 1) * HD].reshape(HD, 1).copy(),
        })
    return in_maps


def kernel(x, Wq, bq, Wk, bk, Wv, bv, Wo, bo, _trace=False):
    x = np.asarray(x, dtype=np.float32)
    nc = _get_nc()
    in_maps = _make_in_maps(
        x,
        np.asarray(Wq, np.float32), np.asarray(bq, np.float32),
        np.asarray(Wk, np.float32), np.asarray(bk, np.float32),
        np.asarray(Wv, np.float32), np.asarray(bv, np.float32),
        np.asarray(Wo, np.float32),
    )
    res = run_bass_kernel_spmd(nc, in_maps, list(range(8)), trace=_trace)
    bo = np.asarray(bo, np.float32)
    out = np.empty((2, S, D), np.float32)
    for b in range(2):
        acc = res.results[b * 4]["out"].astype(np.float32)
        for g in range(1, 4):
            acc = acc + res.results[b * 4 + g]["out"]
        out[b] = acc + bo[None, :]
    if _trace:
        return out, res
    return out



# revision 4
# speedup vs baseline: 1.1576x; 1.1576x over previous
"""GQA attention block on 8 Trainium2 cores.

Sharding: data-parallel over batch B=2 x tensor-parallel over the 4 KV groups
(cores 0-3 -> batch 0 groups 0-3, cores 4-7 -> batch 1 groups 0-3).
Each core computes Q/K/V projections for its group, attention for its 4 query
heads, and a row-sharded partial of the output projection.  The host sums the
4 partials per batch and adds the output bias.

v2 changes vs baseline (393 us):
- All matmul operands in bf16 (same PE rate as fp32r, but halves DMA bytes,
  halves SBUF, and doubles DVE elementwise throughput).  PSUM stays fp32.
- K bias dropped on device (cancels in softmax: q.bk is constant over t);
  V bias folded into the host-side output bias (sum of attn weights == 1).
- Denominator path restructured: exp(tg0/tg4) written straight into two
  persistent accumulators (kills 64 init copies), remaining tiles added with
  wide FD-1024 bf16 adds (6 instead of 12+3 per head), partition reduction
  via 4 accumulating ones-matmuls on the PE (kills the DVE merges).
- reciprocal output bitcast to f32r instead of a cast copy.
- Software pipelining: scores(tg+1) is emitted before attnV(tg) so the PE
  streams ahead of the ACT exp; the previous head's tail (split in two so
  the ps_b matmul never waits on the reciprocal) and the previous chunk's
  out-proj groups are drained into the ACT-bound gaps of the tg loop.
  Tails drain at higher priority than out-proj so PSUM ring slots (bufs=2)
  are always consumed before their WAR reuse two heads later.
"""
import sys

sys.path.insert(0, "/opt/trn_rl_repo")

import math
from collections import deque
from contextlib import ExitStack

import numpy as np
import ml_dtypes

import concourse.bacc as bacc
import concourse.tile as tile
import concourse.mybir as mybir
from concourse.bass_utils import run_bass_kernel_spmd
from concourse.masks import make_identity

F32 = mybir.dt.float32
F32R = mybir.dt.float32r
BF16 = mybir.dt.bfloat16
AF = mybir.ActivationFunctionType

D = 2048          # d_model
S = 2048          # sequence length
HD = 128          # head dim
R = 4             # q heads per kv group (on one core)
GD = R * HD       # 512: q-projection width per core
KT_TILES = S // 128   # 16 key-time tiles
KD_TILES = D // 128   # 16 contraction tiles for projections
N_SC = 4          # s-chunks of 512
SC = S // N_SC    # 512
SCALE = 1.0 / math.sqrt(HD)

_CACHED = {}


def _build():
    nc = bacc.Bacc("TRN2", target_bir_lowering=False, debug=False, num_devices=8)

    XT = nc.dram_tensor("xt", [D, S], BF16, kind="ExternalInput")
    WQ = nc.dram_tensor("wq", [D, GD], BF16, kind="ExternalInput")
    WK = nc.dram_tensor("wk", [D, HD], BF16, kind="ExternalInput")
    WV = nc.dram_tensor("wv", [D, HD], BF16, kind="ExternalInput")
    WO = nc.dram_tensor("wo", [GD, D], BF16, kind="ExternalInput")
    BQ = nc.dram_tensor("bq", [128, R], F32, kind="ExternalInput")
    OUT = nc.dram_tensor("out", [S, D], F32, kind="ExternalOutput")

    with tile.TileContext(nc) as tc, ExitStack() as ctx:
        # ---- long-lived tiles ----
        lp = ctx.enter_context(tc.tile_pool(name="long", bufs=1))
        qt_sb = lp.tile([128, R, S], BF16)        # Q^T per head: [dq, h, s]
        kt_sb = lp.tile([128, S], BF16)           # K^T: [dk, t]
        vt_sb = lp.tile([128, S], F32R)           # V^T: [dv, t]
        v_sb = lp.tile([128, KT_TILES, HD], BF16) # V natural: [t_sub, t_tile, dv]
        bq_sb = lp.tile([128, R], F32)
        ones_col = lp.tile([128, 1], BF16)
        ones_row = lp.tile([1, 128], F32R)
        ident = lp.tile([128, 128], F32R)

        nc.sync.dma_start(bq_sb[:], BQ.ap())

        tmp_f = lp.tile([128, 128], F32)
        nc.gpsimd.memset(tmp_f[:], 1.0)
        nc.vector.tensor_copy(ones_col[:], tmp_f[:, 0:1])
        nc.vector.tensor_copy(ones_row[:], tmp_f[0:1, 0:128])
        make_identity(nc, tmp_f[:])
        nc.vector.tensor_copy(ident[:], tmp_f[:])

        # ---- phase A: projections ----
        with ExitStack() as actx:
            wp = actx.enter_context(tc.tile_pool(name="wqkv", bufs=1))
            xp = actx.enter_context(tc.tile_pool(name="xt", bufs=2))
            psa = actx.enter_context(tc.tile_pool(name="psa", bufs=4, space="PSUM"))

            wq_sb = wp.tile([128, KD_TILES, GD], BF16)
            wk_sb = wp.tile([128, KD_TILES, HD], BF16)
            wv_sb = wp.tile([128, KD_TILES, HD], BF16)
            wq_r = WQ.ap().rearrange("(ko p) n -> p ko n", p=128)
            wk_r = WK.ap().rearrange("(ko p) n -> p ko n", p=128)
            wv_r = WV.ap().rearrange("(ko p) n -> p ko n", p=128)

            for sc in range(N_SC):
                xt = xp.tile([128, KD_TILES, SC], BF16, tag="xt")
                xt_r = XT.ap()[:, sc * SC:(sc + 1) * SC].rearrange(
                    "(ko p) s -> p ko s", p=128
                )
                # interleave per-k so the k=0 matmuls can start almost
                # immediately (weights ride along with the first chunk)
                for k in range(KD_TILES):
                    nc.sync.dma_start(xt[:, k, :], xt_r[:, k, :])
                    if sc == 0:
                        nc.sync.dma_start(wk_sb[:, k, :], wk_r[:, k, :])
                        nc.sync.dma_start(wv_sb[:, k, :], wv_r[:, k, :])
                        nc.sync.dma_start(wq_sb[:, k, :], wq_r[:, k, :])
                # K^T (no bias: it cancels in the softmax)
                ps = psa.tile([128, SC], F32, tag="psa")
                for k in range(KD_TILES):
                    nc.tensor.matmul(
                        ps[:], lhsT=wk_sb[:, k, :], rhs=xt[:, k, :],
                        start=(k == 0), stop=(k == KD_TILES - 1),
                    )
                nc.vector.tensor_copy(kt_sb[:, sc * SC:(sc + 1) * SC], ps[:])
                # V^T (no bias: folded into the host-side output bias)
                ps = psa.tile([128, SC], F32, tag="psa")
                for k in range(KD_TILES):
                    nc.tensor.matmul(
                        ps[:], lhsT=wv_sb[:, k, :], rhs=xt[:, k, :],
                        start=(k == 0), stop=(k == KD_TILES - 1),
                    )
                nc.vector.tensor_copy(vt_sb[:, sc * SC:(sc + 1) * SC], ps[:])
                # Q^T for the 4 heads
                for dq in range(R):
                    ps = psa.tile([128, SC], F32, tag="psa")
                    for k in range(KD_TILES):
                        nc.tensor.matmul(
                            ps[:],
                            lhsT=wq_sb[:, k, dq * 128:(dq + 1) * 128],
                            rhs=xt[:, k, :],
                            start=(k == 0), stop=(k == KD_TILES - 1),
                        )
                    nc.scalar.activation(
                        qt_sb[:, dq, sc * SC:(sc + 1) * SC], ps[:],
                        AF.Identity, bias=bq_sb[:, dq:dq + 1],
                    )

            # V^T -> V natural (16 PE transposes), cast to bf16 on the copy
            pst = actx.enter_context(tc.tile_pool(name="pst", bufs=2, space="PSUM"))
            for t in range(KT_TILES):
                pt_ps = pst.tile([128, 128], F32R, tag="pst")
                nc.tensor.transpose(
                    pt_ps[:], vt_sb[:, t * 128:(t + 1) * 128], ident[:]
                )
                nc.vector.tensor_copy(v_sb[:, t, :], pt_ps[:])

        # ---- phase B: attention + out-proj ----
        with ExitStack() as bctx:
            wop = bctx.enter_context(tc.tile_pool(name="wo", bufs=1))
            wo_sb = wop.tile([128, R, D], BF16)
            nc.sync.dma_start(wo_sb[:], WO.ap().rearrange("(h p) n -> p h n", p=128))

            pss = bctx.enter_context(tc.tile_pool(name="pss", bufs=2, space="PSUM"))
            pso = bctx.enter_context(tc.tile_pool(name="pso", bufs=2, space="PSUM"))
            psm = bctx.enter_context(tc.tile_pool(name="psm", bufs=2, space="PSUM"))
            ptp = bctx.enter_context(tc.tile_pool(name="ptp", bufs=3))
            accp = bctx.enter_context(tc.tile_pool(name="accp", bufs=2))
            otp = bctx.enter_context(tc.tile_pool(name="otp", bufs=2))
            outp = bctx.enter_context(tc.tile_pool(name="outp", bufs=4))

            # closures emitted into the ACT-bound gaps of the tg loop.
            # tail_aux (previous head's tail) has priority over op_aux
            # (previous chunk's out-proj) so PSUM ring slots are always
            # consumed before their WAR reuse two heads later.
            tail_aux = deque()
            op_aux = deque()

            def drain_aux():
                if tail_aux:
                    tail_aux.popleft()()
                elif op_aux:
                    op_aux.popleft()()

            def attn_head(sc, h):
                """scores -> exp -> attnV for one (chunk, head); returns the
                unnormalized PSUM accumulator and the two bf16 denominator
                accumulators."""
                ps_o = pso.tile([128, SC], F32, tag="pso", name="ps_o")
                accA = accp.tile([128, 2, SC], BF16, tag="accA", name="accA")
                accB = accp.tile([128, 2, SC], BF16, tag="accB", name="accB")

                def scores(tg):
                    ps_s = pss.tile([128, 2, SC], F32, tag="pss", name="ps_s")
                    for i in range(2):
                        t = tg * 2 + i
                        nc.tensor.matmul(
                            ps_s[:, i, :],
                            lhsT=kt_sb[:, t * 128:(t + 1) * 128],
                            rhs=qt_sb[:, h, sc * SC:(sc + 1) * SC],
                            start=True, stop=True,
                        )
                    return ps_s

                ps_s = scores(0)
                for tg in range(8):
                    # exp of this pair (tg0/tg4 land in the accumulators)
                    if tg == 0:
                        pt = accA
                    elif tg == 4:
                        pt = accB
                    else:
                        pt = ptp.tile([128, 2, SC], BF16, tag="pt", name="pt")
                    nc.scalar.activation(pt[:], ps_s[:], AF.Exp, scale=SCALE)
                    # stream next pair's scores ahead of the exp consumer
                    if tg < 7:
                        ps_s = scores(tg + 1)
                    # attnV of this pair
                    for i in range(2):
                        t = tg * 2 + i
                        nc.tensor.matmul(
                            ps_o[:],
                            lhsT=v_sb[:, t, :],
                            rhs=pt[:, i, :],
                            start=(t == 0),
                            stop=(t == KT_TILES - 1),
                            skip_group_check=True,
                        )
                    # denominator partials (wide bf16 adds, 2x DVE mode)
                    if tg in (1, 2, 3):
                        nc.vector.tensor_add(accA[:], accA[:], pt[:])
                    elif tg in (5, 6, 7):
                        nc.vector.tensor_add(accB[:], accB[:], pt[:])
                    drain_aux()
                return ps_o, accA, accB

            def tail_parts(h, ot_sb, ps_o, accA, accB):
                """Two closures: denominator+reciprocal, then broadcast+
                normalize (split so ps_b never queues behind the reciprocal)."""
                recip = accp.tile([1, SC], F32, tag="recip", name="recip")
                recip_r = accp.tile([1, SC], F32R, tag="recip_r", name="recip_r")

                def t1():
                    ps_d = psm.tile([1, SC], F32, tag="psm", name="ps_d")
                    rhss = [accA[:, 0, :], accA[:, 1, :],
                            accB[:, 0, :], accB[:, 1, :]]
                    for j, rhs in enumerate(rhss):
                        nc.tensor.matmul(
                            ps_d[:], lhsT=ones_col[:], rhs=rhs,
                            start=(j == 0), stop=(j == 3),
                        )
                    nc.vector.reciprocal_approx_fast(recip[:], ps_d[:])
                    nc.vector.tensor_copy(recip_r[:], recip[:])

                def t2():
                    ps_b = psm.tile([128, SC], F32, tag="psm", name="ps_b")
                    nc.tensor.matmul(
                        ps_b[:], lhsT=ones_row[:], rhs=recip_r[:],
                        start=True, stop=True,
                    )
                    bc = accp.tile([128, SC], F32, tag="bc", name="bc")
                    nc.scalar.copy(bc[:], ps_b[:])
                    nc.vector.tensor_mul(ot_sb[:, h, :], ps_o[:], bc[:])

                return t1, t2

            def out_proj_groups(sc, ot_sb):
                """16 closures, each one PSUM accumulation + store."""
                def group(st, oc):
                    def run():
                        ps_f = psm.tile([128, 512], F32, tag="psm", name="ps_f")
                        for dv in range(R):
                            nc.tensor.matmul(
                                ps_f[:],
                                lhsT=ot_sb[:, dv, st * 128:(st + 1) * 128],
                                rhs=wo_sb[:, dv, oc * 512:(oc + 1) * 512],
                                start=(dv == 0),
                                stop=(dv == R - 1),
                            )
                        o_t = outp.tile([128, 512], F32, tag="out", name="o_t")
                        if (st + oc) % 2 == 0:
                            nc.scalar.copy(o_t[:], ps_f[:])
                        else:
                            nc.vector.tensor_copy(o_t[:], ps_f[:])
                        nc.sync.dma_start(
                            OUT.ap()[
                                sc * SC + st * 128: sc * SC + (st + 1) * 128,
                                oc * 512:(oc + 1) * 512,
                            ],
                            o_t[:],
                        )
                    return run
                return [group(st, oc) for st in range(SC // 128)
                        for oc in range(D // 512)]

            for sc in range(N_SC):
                ot_sb = otp.tile([128, R, SC], BF16, tag="ot", name="ot_sb")
                for h in range(R):
                    ps_o, accA, accB = attn_head(sc, h)
                    t1, t2 = tail_parts(h, ot_sb, ps_o, accA, accB)
                    tail_aux.append(t1)
                    tail_aux.append(t2)
                # previous chunk's out-proj drains during the next chunk
                op_aux.extend(out_proj_groups(sc, ot_sb))
            # flush: last head's tail + last chunk's out-proj
            while tail_aux or op_aux:
                drain_aux()

    nc.compile()
    return nc


def _get_nc():
    if "nc" not in _CACHED:
        _CACHED["nc"] = _build()
    return _CACHED["nc"]


def _make_in_maps(x, Wq, bq, Wk, Wv, Wo):
    bf = ml_dtypes.bfloat16
    in_maps = []
    xts = [np.ascontiguousarray(x[b].T).astype(bf) for b in range(2)]
    wq_b = Wq.astype(bf)
    wk_b = Wk.astype(bf)
    wv_b = Wv.astype(bf)
    wo_b = Wo.astype(bf)
    for core in range(8):
        b, g = divmod(core, 4)
        in_maps.append({
            "xt": xts[b],
            "wq": np.ascontiguousarray(wq_b[:, g * GD:(g + 1) * GD]),
            "wk": np.ascontiguousarray(wk_b[:, g * HD:(g + 1) * HD]),
            "wv": np.ascontiguousarray(wv_b[:, g * HD:(g + 1) * HD]),
            "wo": np.ascontiguousarray(wo_b[g * GD:(g + 1) * GD, :]),
            "bq": np.ascontiguousarray(
                bq[g * GD:(g + 1) * GD].reshape(R, 128).T
            ).astype(np.float32),
        })
    return in_maps


def kernel(x, Wq, bq, Wk, bk, Wv, bv, Wo, bo, _trace=False):
    x = np.asarray(x, dtype=np.float32)
    Wq = np.asarray(Wq, np.float32)
    bq = np.asarray(bq, np.float32)
    Wk = np.asarray(Wk, np.float32)
    Wv = np.asarray(Wv, np.float32)
    Wo = np.asarray(Wo, np.float32)
    bv = np.asarray(bv, np.float32)
    bo = np.asarray(bo, np.float32)
    nc = _get_nc()
    in_maps = _make_in_maps(x, Wq, bq, Wk, Wv, Wo)
    res = run_bass_kernel_spmd(nc, in_maps, list(range(8)), trace=_trace)
    # host-side bias: bo + contribution of the V bias through the out-proj
    # (attention weights sum to 1, so each head adds bv[group] @ Wo_head)
    H = 16
    bias_full = bo.copy()
    for h in range(H):
        g = h // R
        bias_full += bv[g * HD:(g + 1) * HD] @ Wo[h * HD:(h + 1) * HD, :]
    out = np.empty((2, S, D), np.float32)
    for b in range(2):
        acc = res.results[b * 4]["out"].astype(np.float32)
        for g in range(1, 4):
            acc = acc + res.results[b * 4 + g]["out"]
        out[b] = acc + bias_full[None, :]
    if _trace:
        return out, res
    return out


# revision 6
# speedup vs baseline: 1.2634x; 1.0914x over previous
"""GQA attention block on 8 Trainium2 cores.

Sharding: data-parallel over batch B=2 x tensor-parallel over the 4 KV groups
(cores 0-3 -> batch 0 groups 0-3, cores 4-7 -> batch 1 groups 0-3).
Each core computes Q/K/V projections for its group, attention for its 4 query
heads, and a row-sharded partial of the output projection.  The host sums the
4 partials per batch and adds the output bias.

v3 (vs v2 343 us, baseline 393 us):
- Host ships x and the weights pre-tiled partition-major so every transfer
  is one fat DMA with 4-16 KB contiguous per partition (the per-DMA issue
  overhead was gating phase A at ~200 GB/s with 1 KB lines).
- attnV delayed one more pipeline stage (exp(tg) -> scores(tg+1) ->
  attnV(tg-1)) so the PE never in-order-waits on the ACT exp.
- Denominator: accB merged into accA on the DVE, halving the ones-matmuls.
- Out-proj PSUM->SBUF copies all on the DVE (ACT is the phase-B floor).
- V transposes interleaved per chunk.
All matmul operands bf16 (same PE rate as fp32r, half the DMA/SBUF, 2x DVE);
K bias dropped (softmax-invariant), V bias folded into the host output bias.
"""
import sys

sys.path.insert(0, "/opt/trn_rl_repo")

import math
from collections import deque
from contextlib import ExitStack

import numpy as np
import ml_dtypes

import concourse.bacc as bacc
import concourse.tile as tile
import concourse.mybir as mybir
from concourse.bass_utils import run_bass_kernel_spmd
from concourse.masks import make_identity

F32 = mybir.dt.float32
F32R = mybir.dt.float32r
BF16 = mybir.dt.bfloat16
AF = mybir.ActivationFunctionType

D = 2048          # d_model
S = 2048          # sequence length
HD = 128          # head dim
R = 4             # q heads per kv group (on one core)
GD = R * HD       # 512: q-projection width per core
KT_TILES = S // 128   # 16 key-time tiles
KD_TILES = D // 128   # 16 contraction tiles for projections
N_SC = 4          # s-chunks of 512
SC = S // N_SC    # 512
SCALE = 1.0 / math.sqrt(HD)

_CACHED = {}


def _build():
    nc = bacc.Bacc("TRN2", target_bir_lowering=False, debug=False, num_devices=8)

    # all pre-tiled partition-major on the host for contiguous DMA
    XT = nc.dram_tensor("xt", [128, N_SC, KD_TILES, SC], BF16, kind="ExternalInput")
    WQ = nc.dram_tensor("wq", [128, KD_TILES, GD], BF16, kind="ExternalInput")
    WK = nc.dram_tensor("wk", [128, KD_TILES, HD], BF16, kind="ExternalInput")
    WV = nc.dram_tensor("wv", [128, KD_TILES, HD], BF16, kind="ExternalInput")
    WO = nc.dram_tensor("wo", [128, R, D], BF16, kind="ExternalInput")
    BQ = nc.dram_tensor("bq", [128, R], F32, kind="ExternalInput")
    OUT = nc.dram_tensor("out", [S, D], F32, kind="ExternalOutput")

    with tile.TileContext(nc) as tc, ExitStack() as ctx:
        # ---- long-lived tiles ----
        lp = ctx.enter_context(tc.tile_pool(name="long", bufs=1))
        qt_sb = lp.tile([128, R, S], BF16)        # Q^T per head: [dq, h, s]
        kt_sb = lp.tile([128, S], BF16)           # K^T: [dk, t]
        vt_sb = lp.tile([128, S], F32R)           # V^T: [dv, t]
        v_sb = lp.tile([128, KT_TILES, HD], BF16) # V natural: [t_sub, t_tile, dv]
        bq_sb = lp.tile([128, R], F32)
        ones_col = lp.tile([128, 1], BF16)
        ones_row = lp.tile([1, 128], F32R)
        ident = lp.tile([128, 128], F32R)

        nc.scalar.dma_start(bq_sb[:], BQ.ap())

        tmp_f = lp.tile([128, 128], F32)
        nc.gpsimd.memset(tmp_f[:], 1.0)
        nc.vector.tensor_copy(ones_col[:], tmp_f[:, 0:1])
        nc.vector.tensor_copy(ones_row[:], tmp_f[0:1, 0:128])
        make_identity(nc, tmp_f[:])
        nc.vector.tensor_copy(ident[:], tmp_f[:])

        # ---- phase A: projections ----
        with ExitStack() as actx:
            wp = actx.enter_context(tc.tile_pool(name="wqkv", bufs=1))
            xp = actx.enter_context(tc.tile_pool(name="xt", bufs=2))
            psa = actx.enter_context(tc.tile_pool(name="psa", bufs=4, space="PSUM"))
            pst = actx.enter_context(tc.tile_pool(name="pst", bufs=2, space="PSUM"))

            wq_sb = wp.tile([128, KD_TILES, GD], BF16)
            wk_sb = wp.tile([128, KD_TILES, HD], BF16)
            wv_sb = wp.tile([128, KD_TILES, HD], BF16)

            # fat contiguous DMAs; K/V weights first so their matmuls can
            # chase the first x chunk
            nc.sync.dma_start(wk_sb[:], WK.ap())
            nc.sync.dma_start(wv_sb[:], WV.ap())

            for sc in range(N_SC):
                xt = xp.tile([128, KD_TILES, SC], BF16, tag="xt")
                if sc == 0:
                    # split so the k=0 matmuls start after ~1/4 chunk
                    for kg in range(4):
                        nc.sync.dma_start(
                            xt[:, kg * 4:(kg + 1) * 4, :],
                            XT.ap()[:, 0, kg * 4:(kg + 1) * 4, :],
                        )
                    nc.sync.dma_start(wq_sb[:], WQ.ap())
                else:
                    nc.sync.dma_start(xt[:], XT.ap()[:, sc, :, :])
                # K^T (no bias: it cancels in the softmax)
                ps = psa.tile([128, SC], F32, tag="psa")
                for k in range(KD_TILES):
                    nc.tensor.matmul(
                        ps[:], lhsT=wk_sb[:, k, :], rhs=xt[:, k, :],
                        start=(k == 0), stop=(k == KD_TILES - 1),
                    )
                nc.vector.tensor_copy(kt_sb[:, sc * SC:(sc + 1) * SC], ps[:])
                # V^T (no bias: folded into the host-side output bias)
                ps = psa.tile([128, SC], F32, tag="psa")
                for k in range(KD_TILES):
                    nc.tensor.matmul(
                        ps[:], lhsT=wv_sb[:, k, :], rhs=xt[:, k, :],
                        start=(k == 0), stop=(k == KD_TILES - 1),
                    )
                nc.vector.tensor_copy(vt_sb[:, sc * SC:(sc + 1) * SC], ps[:])
                # Q^T for the 4 heads
                for dq in range(R):
                    ps = psa.tile([128, SC], F32, tag="psa")
                    for k in range(KD_TILES):
                        nc.tensor.matmul(
                            ps[:],
                            lhsT=wq_sb[:, k, dq * 128:(dq + 1) * 128],
                            rhs=xt[:, k, :],
                            start=(k == 0), stop=(k == KD_TILES - 1),
                        )
                    nc.scalar.activation(
                        qt_sb[:, dq, sc * SC:(sc + 1) * SC], ps[:],
                        AF.Identity, bias=bq_sb[:, dq:dq + 1],
                    )
                # V^T -> V natural for this chunk (4 PE transposes)
                for t in range(sc * 4, sc * 4 + 4):
                    pt_ps = pst.tile([128, 128], F32R, tag="pst")
                    nc.tensor.transpose(
                        pt_ps[:], vt_sb[:, t * 128:(t + 1) * 128], ident[:]
                    )
                    nc.vector.tensor_copy(v_sb[:, t, :], pt_ps[:])

        # ---- phase B: attention + out-proj ----
        with ExitStack() as bctx:
            wop = bctx.enter_context(tc.tile_pool(name="wo", bufs=1))
            wo_sb = wop.tile([128, R, D], BF16)
            nc.sync.dma_start(wo_sb[:], WO.ap())

            pss = bctx.enter_context(tc.tile_pool(name="pss", bufs=2, space="PSUM"))
            pso = bctx.enter_context(tc.tile_pool(name="pso", bufs=2, space="PSUM"))
            psm = bctx.enter_context(tc.tile_pool(name="psm", bufs=2, space="PSUM"))
            ptp = bctx.enter_context(tc.tile_pool(name="ptp", bufs=3))
            accp = bctx.enter_context(tc.tile_pool(name="accp", bufs=2))
            otp = bctx.enter_context(tc.tile_pool(name="otp", bufs=2))
            outp = bctx.enter_context(tc.tile_pool(name="outp", bufs=4))

            # closures emitted into the gaps of the tg loop; tails have
            # priority so PSUM ring slots (bufs=2) are always consumed
            # before their WAR reuse two heads later
            tail_aux = deque()
            op_aux = deque()

            def drain_aux():
                if tail_aux:
                    tail_aux.popleft()()
                elif op_aux:
                    op_aux.popleft()()

            def attn_head(sc, h):
                ps_o = pso.tile([128, SC], F32, tag="pso", name="ps_o")
                accA = accp.tile([128, 2, SC], BF16, tag="accA", name="accA")
                accB = accp.tile([128, 2, SC], BF16, tag="accB", name="accB")

                def scores(tg):
                    ps_s = pss.tile([128, 2, SC], F32, tag="pss", name="ps_s")
                    for i in range(2):
                        t = tg * 2 + i
                        nc.tensor.matmul(
                            ps_s[:, i, :],
                            lhsT=kt_sb[:, t * 128:(t + 1) * 128],
                            rhs=qt_sb[:, h, sc * SC:(sc + 1) * SC],
                            start=True, stop=True,
                        )
                    return ps_s

                def attnv(tg, pt):
                    for i in range(2):
                        t = tg * 2 + i
                        nc.tensor.matmul(
                            ps_o[:],
                            lhsT=v_sb[:, t, :],
                            rhs=pt[:, i, :],
                            start=(t == 0),
                            stop=(t == KT_TILES - 1),
                            skip_group_check=True,
                        )

                ps_s = scores(0)
                pt_hist = {}
                for tg in range(8):
                    # exp of this pair (tg0/tg4 land in the accumulators)
                    if tg == 0:
                        pt = accA
                    elif tg == 4:
                        pt = accB
                    else:
                        pt = ptp.tile([128, 2, SC], BF16, tag="pt", name="pt")
                    nc.scalar.activation(pt[:], ps_s[:], AF.Exp, scale=SCALE)
                    pt_hist[tg] = pt
                    # stream next pair's scores ahead of the exp consumer
                    if tg < 7:
                        ps_s = scores(tg + 1)
                    # attnV delayed one stage: consumes exp finished a full
                    # period ago, so the PE never waits on the ACT here
                    if tg >= 1:
                        attnv(tg - 1, pt_hist[tg - 1])
                    # denominator partials (wide bf16 adds, 2x DVE mode)
                    if tg in (1, 2, 3):
                        nc.vector.tensor_add(accA[:], accA[:], pt[:])
                    elif tg in (5, 6, 7):
                        nc.vector.tensor_add(accB[:], accB[:], pt[:])
                    drain_aux()
                attnv(7, pt_hist[7])
                return ps_o, accA, accB

            def tail_parts(h, ot_sb, ps_o, accA, accB):
                """Two closures: denominator+reciprocal, then broadcast+
                normalize (split so ps_b never queues behind the reciprocal)."""
                recip = accp.tile([1, SC], F32, tag="recip", name="recip")
                recip_r = accp.tile([1, SC], F32R, tag="recip_r", name="recip_r")

                def t1():
                    nc.vector.tensor_add(accA[:], accA[:], accB[:])
                    ps_d = psm.tile([1, SC], F32, tag="psm", name="ps_d")
                    for j in range(2):
                        nc.tensor.matmul(
                            ps_d[:], lhsT=ones_col[:], rhs=accA[:, j, :],
                            start=(j == 0), stop=(j == 1),
                        )
                    nc.vector.reciprocal_approx_fast(recip[:], ps_d[:])
                    nc.vector.tensor_copy(recip_r[:], recip[:])

                def t2():
                    ps_b = psm.tile([128, SC], F32, tag="psm", name="ps_b")
                    nc.tensor.matmul(
                        ps_b[:], lhsT=ones_row[:], rhs=recip_r[:],
                        start=True, stop=True,
                    )
                    bc = accp.tile([128, SC], F32, tag="bc", name="bc")
                    nc.scalar.copy(bc[:], ps_b[:])
                    nc.vector.tensor_mul(ot_sb[:, h, :], ps_o[:], bc[:])

                return t1, t2

            def out_proj_groups(sc, ot_sb):
                """16 closures, each one PSUM accumulation + store."""
                def group(st, oc):
                    def run():
                        ps_f = psm.tile([128, 512], F32, tag="psm", name="ps_f")
                        for dv in range(R):
                            nc.tensor.matmul(
                                ps_f[:],
                                lhsT=ot_sb[:, dv, st * 128:(st + 1) * 128],
                                rhs=wo_sb[:, dv, oc * 512:(oc + 1) * 512],
                                start=(dv == 0),
                                stop=(dv == R - 1),
                            )
                        o_t = outp.tile([128, 512], F32, tag="out", name="o_t")
                        nc.vector.tensor_copy(o_t[:], ps_f[:])
                        nc.sync.dma_start(
                            OUT.ap()[
                                sc * SC + st * 128: sc * SC + (st + 1) * 128,
                                oc * 512:(oc + 1) * 512,
                            ],
                            o_t[:],
                        )
                    return run
                return [group(st, oc) for st in range(SC // 128)
                        for oc in range(D // 512)]

            for sc in range(N_SC):
                ot_sb = otp.tile([128, R, SC], BF16, tag="ot", name="ot_sb")
                for h in range(R):
                    ps_o, accA, accB = attn_head(sc, h)
                    t1, t2 = tail_parts(h, ot_sb, ps_o, accA, accB)
                    tail_aux.append(t1)
                    tail_aux.append(t2)
                # previous chunk's out-proj drains during the next chunk
                op_aux.extend(out_proj_groups(sc, ot_sb))
            # flush: last head's tail + last chunk's out-proj
            while tail_aux or op_aux:
                drain_aux()

    nc.compile()
    return nc


def _get_nc():
    if "nc" not in _CACHED:
        _CACHED["nc"] = _build()
    return _CACHED["nc"]


def _tile_p(a, nt, width):
    """[nt*128, width] -> [128, nt, width] partition-major."""
    return np.ascontiguousarray(
        a.reshape(nt, 128, width).transpose(1, 0, 2)
    )


def _make_in_maps(x, Wq, bq, Wk, Wv, Wo):
    bf = ml_dtypes.bfloat16
    in_maps = []
    xts = []
    for b in range(2):
        xt = np.ascontiguousarray(x[b].T).astype(bf)      # [D, S]
        # [D, S] -> [128, N_SC, KD, SC]
        xt = xt.reshape(KD_TILES, 128, N_SC, SC).transpose(1, 2, 0, 3)
        xts.append(np.ascontiguousarray(xt))
    wq_b = Wq.astype(bf)
    wk_b = Wk.astype(bf)
    wv_b = Wv.astype(bf)
    wo_b = Wo.astype(bf)
    for core in range(8):
        b, g = divmod(core, 4)
        in_maps.append({
            "xt": xts[b],
            "wq": _tile_p(wq_b[:, g * GD:(g + 1) * GD], KD_TILES, GD),
            "wk": _tile_p(wk_b[:, g * HD:(g + 1) * HD], KD_TILES, HD),
            "wv": _tile_p(wv_b[:, g * HD:(g + 1) * HD], KD_TILES, HD),
            "wo": _tile_p(wo_b[g * GD:(g + 1) * GD, :], R, D),
            "bq": np.ascontiguousarray(
                bq[g * GD:(g + 1) * GD].reshape(R, 128).T
            ).astype(np.float32),
        })
    return in_maps


def kernel(x, Wq, bq, Wk, bk, Wv, bv, Wo, bo, _trace=False):
    x = np.asarray(x, dtype=np.float32)
    Wq = np.asarray(Wq, np.float32)
    bq = np.asarray(bq, np.float32)
    Wk = np.asarray(Wk, np.float32)
    Wv = np.asarray(Wv, np.float32)
    Wo = np.asarray(Wo, np.float32)
    bv = np.asarray(bv, np.float32)
    bo = np.asarray(bo, np.float32)
    nc = _get_nc()
    in_maps = _make_in_maps(x, Wq, bq, Wk, Wv, Wo)
    res = run_bass_kernel_spmd(nc, in_maps, list(range(8)), trace=_trace)
    # host-side bias: bo + contribution of the V bias through the out-proj
    # (attention weights sum to 1, so each head adds bv[group] @ Wo_head)
    H = 16
    bias_full = bo.copy()
    for h in range(H):
        g = h // R
        bias_full += bv[g * HD:(g + 1) * HD] @ Wo[h * HD:(h + 1) * HD, :]
    out = np.empty((2, S, D), np.float32)
    for b in range(2):
        acc = res.results[b * 4]["out"].astype(np.float32)
        for g in range(1, 4):
            acc = acc + res.results[b * 4 + g]["out"]
        out[b] = acc + bias_full[None, :]
    if _trace:
        return out, res
    return out


# revision 12
# speedup vs baseline: 1.3014x; 1.0301x over previous
"""GQA attention block on 8 Trainium2 cores.

Sharding: data-parallel over batch B=2 x tensor-parallel over the 4 KV groups
(cores 0-3 -> batch 0 groups 0-3, cores 4-7 -> batch 1 groups 0-3).
Each core computes Q/K/V projections for its group, attention for its 4 query
heads, and a row-sharded partial of the output projection.  The host sums the
4 partials per batch and adds the output bias.

v3 (vs v2 343 us, baseline 393 us):
- Host ships x and the weights pre-tiled partition-major so every transfer
  is one fat DMA with 4-16 KB contiguous per partition (the per-DMA issue
  overhead was gating phase A at ~200 GB/s with 1 KB lines).
- attnV delayed one more pipeline stage (exp(tg) -> scores(tg+1) ->
  attnV(tg-1)) so the PE never in-order-waits on the ACT exp.
- Denominator: accB merged into accA on the DVE, halving the ones-matmuls.
- Out-proj PSUM->SBUF copies all on the DVE (ACT is the phase-B floor).
- V transposes interleaved per chunk.
All matmul operands bf16 (same PE rate as fp32r, half the DMA/SBUF, 2x DVE);
K bias dropped (softmax-invariant), V bias folded into the host output bias.
"""
import sys

sys.path.insert(0, "/opt/trn_rl_repo")

import math
from collections import deque
from contextlib import ExitStack

import numpy as np
import ml_dtypes

import concourse.bacc as bacc
import concourse.tile as tile
import concourse.mybir as mybir
from concourse.bass_utils import run_bass_kernel_spmd
from concourse.masks import make_identity

F32 = mybir.dt.float32
F32R = mybir.dt.float32r
BF16 = mybir.dt.bfloat16
AF = mybir.ActivationFunctionType

D = 2048          # d_model
S = 2048          # sequence length
HD = 128          # head dim
R = 4             # q heads per kv group (on one core)
GD = R * HD       # 512: q-projection width per core
KT_TILES = S // 128   # 16 key-time tiles
KD_TILES = D // 128   # 16 contraction tiles for projections
N_SC = 4          # s-chunks of 512
SC = S // N_SC    # 512
SCALE = 1.0 / math.sqrt(HD)

_CACHED = {}


def _build():
    nc = bacc.Bacc("TRN2", target_bir_lowering=False, debug=False, num_devices=8)

    # all pre-tiled partition-major on the host for contiguous DMA
    XT = nc.dram_tensor("xt", [128, N_SC, KD_TILES, SC], BF16, kind="ExternalInput")
    WQ = nc.dram_tensor("wq", [128, KD_TILES, GD], BF16, kind="ExternalInput")
    WK = nc.dram_tensor("wk", [128, KD_TILES, HD], BF16, kind="ExternalInput")
    WV = nc.dram_tensor("wv", [128, KD_TILES, HD], BF16, kind="ExternalInput")
    WO = nc.dram_tensor("wo", [128, R, D], BF16, kind="ExternalInput")
    BQ = nc.dram_tensor("bq", [128, R], F32, kind="ExternalInput")
    OUT = nc.dram_tensor("out", [S, D], F32, kind="ExternalOutput")

    with tile.TileContext(nc) as tc, ExitStack() as ctx:
        # ---- long-lived tiles ----
        lp = ctx.enter_context(tc.tile_pool(name="long", bufs=1))
        qt_sb = lp.tile([128, R, S], BF16)        # Q^T per head: [dq, h, s]
        kt_sb = lp.tile([128, S], BF16)           # K^T: [dk, t]
        vt_sb = lp.tile([128, S], F32R)           # V^T: [dv, t]
        v_sb = lp.tile([128, KT_TILES, HD], BF16) # V natural: [t_sub, t_tile, dv]
        bq_sb = lp.tile([128, R], F32)
        ones_col = lp.tile([128, 1], BF16)
        ones_row = lp.tile([1, 128], F32R)
        ident = lp.tile([128, 128], F32R)

        nc.scalar.dma_start(bq_sb[:], BQ.ap())

        tmp_f = lp.tile([128, 128], F32)
        nc.gpsimd.memset(tmp_f[:], 1.0)
        nc.vector.tensor_copy(ones_col[:], tmp_f[:, 0:1])
        nc.vector.tensor_copy(ones_row[:], tmp_f[0:1, 0:128])
        make_identity(nc, tmp_f[:])
        nc.vector.tensor_copy(ident[:], tmp_f[:])

        # ---- phase A: projections ----
        with ExitStack() as actx:
            wp = actx.enter_context(tc.tile_pool(name="wqkv", bufs=1))
            xp = actx.enter_context(tc.tile_pool(name="xt", bufs=2))
            psa = actx.enter_context(tc.tile_pool(name="psa", bufs=4, space="PSUM"))
            pst = actx.enter_context(tc.tile_pool(name="pst", bufs=2, space="PSUM"))

            wq_sb = wp.tile([128, KD_TILES, GD], BF16)
            wk_sb = wp.tile([128, KD_TILES, HD], BF16)
            wv_sb = wp.tile([128, KD_TILES, HD], BF16)

            # fat contiguous DMAs; K/V weights first so their matmuls can
            # chase the first x chunk
            nc.sync.dma_start(wk_sb[:], WK.ap())
            nc.sync.dma_start(wv_sb[:], WV.ap())

            for sc in range(N_SC):
                xt = xp.tile([128, KD_TILES, SC], BF16, tag="xt")
                if sc == 0:
                    # split so the k=0 matmuls start after ~1/4 chunk
                    for kg in range(4):
                        nc.sync.dma_start(
                            xt[:, kg * 4:(kg + 1) * 4, :],
                            XT.ap()[:, 0, kg * 4:(kg + 1) * 4, :],
                        )
                    nc.sync.dma_start(wq_sb[:], WQ.ap())
                else:
                    nc.sync.dma_start(xt[:], XT.ap()[:, sc, :, :])
                # K^T (no bias: it cancels in the softmax)
                ps = psa.tile([128, SC], F32, tag="psa")
                for k in range(KD_TILES):
                    nc.tensor.matmul(
                        ps[:], lhsT=wk_sb[:, k, :], rhs=xt[:, k, :],
                        start=(k == 0), stop=(k == KD_TILES - 1),
                    )
                nc.vector.tensor_copy(kt_sb[:, sc * SC:(sc + 1) * SC], ps[:])
                # V^T (no bias: folded into the host-side output bias)
                ps = psa.tile([128, SC], F32, tag="psa")
                for k in range(KD_TILES):
                    nc.tensor.matmul(
                        ps[:], lhsT=wv_sb[:, k, :], rhs=xt[:, k, :],
                        start=(k == 0), stop=(k == KD_TILES - 1),
                    )
                nc.vector.tensor_copy(vt_sb[:, sc * SC:(sc + 1) * SC], ps[:])
                # Q^T for the 4 heads
                for dq in range(R):
                    ps = psa.tile([128, SC], F32, tag="psa")
                    for k in range(KD_TILES):
                        nc.tensor.matmul(
                            ps[:],
                            lhsT=wq_sb[:, k, dq * 128:(dq + 1) * 128],
                            rhs=xt[:, k, :],
                            start=(k == 0), stop=(k == KD_TILES - 1),
                        )
                    nc.scalar.activation(
                        qt_sb[:, dq, sc * SC:(sc + 1) * SC], ps[:],
                        AF.Identity, bias=bq_sb[:, dq:dq + 1],
                    )
                # V^T -> V natural for this chunk (4 PE transposes)
                for t in range(sc * 4, sc * 4 + 4):
                    pt_ps = pst.tile([128, 128], F32R, tag="pst")
                    nc.tensor.transpose(
                        pt_ps[:], vt_sb[:, t * 128:(t + 1) * 128], ident[:]
                    )
                    nc.vector.tensor_copy(v_sb[:, t, :], pt_ps[:])

        # ---- phase B: attention + out-proj ----
        with ExitStack() as bctx:
            wop = bctx.enter_context(tc.tile_pool(name="wo", bufs=1))
            wo_sb = wop.tile([128, R, D], BF16)
            nc.sync.dma_start(wo_sb[:], WO.ap())

            pss = bctx.enter_context(tc.tile_pool(name="pss", bufs=2, space="PSUM"))
            pso = bctx.enter_context(tc.tile_pool(name="pso", bufs=2, space="PSUM"))
            psm = bctx.enter_context(tc.tile_pool(name="psm", bufs=2, space="PSUM"))
            ptp = bctx.enter_context(tc.tile_pool(name="ptp", bufs=3))
            accp = bctx.enter_context(tc.tile_pool(name="accp", bufs=2))
            otp = bctx.enter_context(tc.tile_pool(name="otp", bufs=2))
            outp = bctx.enter_context(tc.tile_pool(name="outp", bufs=4))

            # closures emitted into the gaps of the tg loop; tails have
            # priority so PSUM ring slots (bufs=2) are always consumed
            # before their WAR reuse two heads later
            tail_aux = deque()
            op_aux = deque()       # holds (run_a, run_b) pairs
            pending = deque()      # forced continuation: run_b right after
                                   # its run_a so no tail can interleave a
                                   # psm allocation between the two halves

            def drain_aux(n=1):
                for _ in range(n):
                    if pending:
                        pending.popleft()()
                    elif tail_aux:
                        tail_aux.popleft()()
                    elif op_aux:
                        a, b = op_aux.popleft()
                        a()
                        pending.append(b)

            def tail_parts(h, ot_sb, ps_o, accA, accB):
                """Two closures: denominator+reciprocal, then broadcast+
                normalize (split so ps_b never queues behind the reciprocal)."""
                recip = accp.tile([1, SC], F32, tag="recip", name="recip")
                recip_r = accp.tile([1, SC], F32R, tag="recip_r", name="recip_r")

                def t1():
                    nc.vector.tensor_add(accA[:], accA[:], accB[:])
                    ps_d = psm.tile([1, SC], F32, tag="psm", name="ps_d")
                    for j in range(2):
                        nc.tensor.matmul(
                            ps_d[:], lhsT=ones_col[:], rhs=accA[:, j, :],
                            start=(j == 0), stop=(j == 1),
                        )
                    nc.vector.reciprocal_approx_fast(recip[:], ps_d[:])
                    nc.vector.tensor_copy(recip_r[:], recip[:])

                def t2():
                    ps_b = psm.tile([128, SC], F32, tag="psm", name="ps_b")
                    nc.tensor.matmul(
                        ps_b[:], lhsT=ones_row[:], rhs=recip_r[:],
                        start=True, stop=True,
                    )
                    bc = accp.tile([128, SC], F32, tag="bc", name="bc")
                    nc.scalar.copy(bc[:], ps_b[:])
                    nc.vector.tensor_mul(ot_sb[:, h, :], ps_o[:], bc[:])

                return t1, t2

            def out_proj_groups(sc, ot_sb):
                """32 half-closures (2 matmuls each) so aux work interleaves
                smoothly instead of clumping into ACT-starving bursts."""
                def group(st, oc):
                    box = [None]

                    def run_a():
                        ps_f = psm.tile([128, 512], F32, tag="psm", name="ps_f")
                        box[0] = ps_f
                        for dv in range(2):
                            nc.tensor.matmul(
                                ps_f[:],
                                lhsT=ot_sb[:, dv, st * 128:(st + 1) * 128],
                                rhs=wo_sb[:, dv, oc * 512:(oc + 1) * 512],
                                start=(dv == 0), stop=False,
                            )

                    def run_b():
                        ps_f = box[0]
                        for dv in range(2, R):
                            nc.tensor.matmul(
                                ps_f[:],
                                lhsT=ot_sb[:, dv, st * 128:(st + 1) * 128],
                                rhs=wo_sb[:, dv, oc * 512:(oc + 1) * 512],
                                start=False, stop=(dv == R - 1),
                            )
                        o_t = outp.tile([128, 512], F32, tag="out", name="o_t")
                        nc.vector.tensor_copy(o_t[:], ps_f[:])
                        nc.sync.dma_start(
                            OUT.ap()[
                                sc * SC + st * 128: sc * SC + (st + 1) * 128,
                                oc * 512:(oc + 1) * 512,
                            ],
                            o_t[:],
                        )
                    return run_a, run_b
                return [group(st, oc) for st in range(SC // 128)
                        for oc in range(D // 512)]

            # ---- one continuous software-pipelined stream over all
            # (chunk, head) units: exp(u,tg) -> scores(u,tg+1 or u+1,0) ->
            # attnV one stage behind.  No per-head fill/drain transients.
            units = [(sc, h) for sc in range(N_SC) for h in range(R)]
            ot_tiles = {}

            def unit_scores(u, tg):
                sc, h = units[u]
                ps_s = pss.tile([128, 2, SC], F32, tag="pss", name="ps_s")
                for i in range(2):
                    t = tg * 2 + i
                    nc.tensor.matmul(
                        ps_s[:, i, :],
                        lhsT=kt_sb[:, t * 128:(t + 1) * 128],
                        rhs=qt_sb[:, h, sc * SC:(sc + 1) * SC],
                        start=True, stop=True,
                    )
                return ps_s

            state = {}   # u -> dict(ps_o, accA, accB, pt_hist)
            prev = None  # (u, tg) of the attnV stage lagging one behind

            def emit_attnv(u, tg):
                st_ = state[u]
                pt = st_["pt_hist"][tg]
                for i in range(2):
                    t = tg * 2 + i
                    nc.tensor.matmul(
                        st_["ps_o"][:],
                        lhsT=v_sb[:, t, :],
                        rhs=pt[:, i, :],
                        start=(t == 0),
                        stop=(t == KT_TILES - 1),
                        skip_group_check=True,
                    )

            ps_s = unit_scores(0, 0)
            for u, (sc, h) in enumerate(units):
                if h == 0:
                    ot_tiles[sc] = otp.tile([128, R, SC], BF16, tag="ot",
                                            name="ot_sb")
                st_ = {
                    "ps_o": pso.tile([128, SC], F32, tag="pso", name="ps_o"),
                    "accA": accp.tile([128, 2, SC], BF16, tag="accA", name="accA"),
                    "accB": accp.tile([128, 2, SC], BF16, tag="accB", name="accB"),
                    "pt_hist": {},
                }
                state[u] = st_
                for tg in range(8):
                    if tg == 0:
                        pt = st_["accA"]
                    elif tg == 4:
                        pt = st_["accB"]
                    else:
                        pt = ptp.tile([128, 2, SC], BF16, tag="pt", name="pt")
                    nc.scalar.activation(pt[:], ps_s[:], AF.Exp, scale=SCALE)
                    st_["pt_hist"][tg] = pt
                    # next position's scores (rolls into the next unit)
                    if tg < 7:
                        ps_s = unit_scores(u, tg + 1)
                    elif u + 1 < len(units):
                        ps_s = unit_scores(u + 1, 0)
                    # attnV lags one stage: never in-order-waits on the ACT
                    if prev is not None:
                        emit_attnv(*prev)
                        if prev[1] == 7:
                            # previous unit complete: queue its tail
                            pu = prev[0]
                            psc, ph = units[pu]
                            t1, t2 = tail_parts(
                                ph, ot_tiles[psc], state[pu]["ps_o"],
                                state[pu]["accA"], state[pu]["accB"],
                            )
                            tail_aux.append(t1)
                            tail_aux.append(t2)
                            del state[pu]["pt_hist"]
                    # denominator partials (wide bf16 adds, 2x DVE mode)
                    if tg in (1, 2, 3):
                        nc.vector.tensor_add(st_["accA"][:], st_["accA"][:],
                                             pt[:])
                    elif tg in (5, 6, 7):
                        nc.vector.tensor_add(st_["accB"][:], st_["accB"][:],
                                             pt[:])
                    prev = (u, tg)
                    drain_aux(2 if tg in (2, 5) else 1)
                if h == R - 1:
                    op_aux.extend(out_proj_groups(sc, ot_tiles[sc]))
            # flush: the trailing attnV, last tail, last chunk's out-proj
            emit_attnv(*prev)
            sc, h = units[-1]
            t1, t2 = tail_parts(h, ot_tiles[sc], state[len(units) - 1]["ps_o"],
                                state[len(units) - 1]["accA"],
                                state[len(units) - 1]["accB"])
            tail_aux.append(t1)
            tail_aux.append(t2)
            while pending or tail_aux or op_aux:
                drain_aux()

    nc.compile()
    return nc


def _get_nc():
    if "nc" not in _CACHED:
        _CACHED["nc"] = _build()
    return _CACHED["nc"]


def _tile_p(a, nt, width):
    """[nt*128, width] -> [128, nt, width] partition-major."""
    return np.ascontiguousarray(
        a.reshape(nt, 128, width).transpose(1, 0, 2)
    )


def _make_in_maps(x, Wq, bq, Wk, Wv, Wo):
    bf = ml_dtypes.bfloat16
    in_maps = []
    xts = []
    for b in range(2):
        xt = np.ascontiguousarray(x[b].T).astype(bf)      # [D, S]
        # [D, S] -> [128, N_SC, KD, SC]
        xt = xt.reshape(KD_TILES, 128, N_SC, SC).transpose(1, 2, 0, 3)
        xts.append(np.ascontiguousarray(xt))
    wq_b = Wq.astype(bf)
    wk_b = Wk.astype(bf)
    wv_b = Wv.astype(bf)
    wo_b = Wo.astype(bf)
    for core in range(8):
        b, g = divmod(core, 4)
        in_maps.append({
            "xt": xts[b],
            "wq": _tile_p(wq_b[:, g * GD:(g + 1) * GD], KD_TILES, GD),
            "wk": _tile_p(wk_b[:, g * HD:(g + 1) * HD], KD_TILES, HD),
            "wv": _tile_p(wv_b[:, g * HD:(g + 1) * HD], KD_TILES, HD),
            "wo": _tile_p(wo_b[g * GD:(g + 1) * GD, :], R, D),
            "bq": np.ascontiguousarray(
                bq[g * GD:(g + 1) * GD].reshape(R, 128).T
            ).astype(np.float32),
        })
    return in_maps


def kernel(x, Wq, bq, Wk, bk, Wv, bv, Wo, bo, _trace=False):
    x = np.asarray(x, dtype=np.float32)
    Wq = np.asarray(Wq, np.float32)
    bq = np.asarray(bq, np.float32)
    Wk = np.asarray(Wk, np.float32)
    Wv = np.asarray(Wv, np.float32)
    Wo = np.asarray(Wo, np.float32)
    bv = np.asarray(bv, np.float32)
    bo = np.asarray(bo, np.float32)
    nc = _get_nc()
    in_maps = _make_in_maps(x, Wq, bq, Wk, Wv, Wo)
    res = run_bass_kernel_spmd(nc, in_maps, list(range(8)), trace=_trace)
    # host-side bias: bo + contribution of the V bias through the out-proj
    # (attention weights sum to 1, so each head adds bv[group] @ Wo_head)
    H = 16
    bias_full = bo.copy()
    for h in range(H):
        g = h // R
        bias_full += bv[g * HD:(g + 1) * HD] @ Wo[h * HD:(h + 1) * HD, :]
    out = np.empty((2, S, D), np.float32)
    for b in range(2):
        acc = res.results[b * 4]["out"].astype(np.float32)
        for g in range(1, 4):
            acc = acc + res.results[b * 4 + g]["out"]
        out[b] = acc + bias_full[None, :]
    if _trace:
        return out, res
    return out
